# revision 7
# baseline (speedup 1.0000x reference)
"""Fused attention + FC + residual + LayerNorm for Trainium2, 8 NeuronCores.

Problem: B=8, L=2048, d_k=d_v=64, d_model=1024, fp32.
Sharding: pure data parallel — batch element b -> core b. No collectives.

Per-core pipeline (all on one NeuronCore, Tile-scheduled):
  1. PE pair-transposes build qT/kT [64, 2048] and fc_wT [65, 1024] (row 64 = fc_b).
  2. For each q-slice of 512 (4 slices), loop k-tile pairs (8 groups):
     S^T tile [128k, 2x512q] on PE -> exp on ScalarE (scale=1/8 folded in)
     -> PV matmul accumulates [65, 512] (row 64 = softmax denominator via
     ones-column appended to V).
  3. Softmax normalization: denominator row is round-tripped through DRAM to
     get a per-partition layout, reciprocal on DVE, broadcast-DMA'd back, and
     fused into the PSUM->SBUF copy as one tensor_tensor multiply. The row-64
     "ones" row for the FC bias augmentation falls out of denom*recip = 1.
  4. FC matmul (K=65 incl. bias row), DVE adds residual from PSUM,
     bn_stats/bn_aggr for LN stats, rsqrt via Log/Exp on ScalarE (same ACT
     table set as the attention exp), LN apply on GPSIMD.
"""
import numpy as np

B = 8
L = 2048
D = 64
DM = 1024
NTILES = L // 128       # 16 q/k tiles of 128
NSLICES = L // 512      # 4 q-slices of 512
LN_EPS = 1e-5
SCALE = 0.125           # 1/sqrt(64)

_CACHE = {}


def _build(affine: bool):
    import concourse.bacc as bacc
    import concourse.tile as tile
    from concourse import mybir
    import concourse.bass as bass
    from concourse.masks import make_identity

    f32 = mybir.dt.float32
    nc = bacc.Bacc("TRN2", target_bir_lowering=False, debug=False, num_devices=B)

    q_d = nc.declare_dram_parameter("q", [L, D], f32, isOutput=False)
    k_d = nc.declare_dram_parameter("k", [L, D], f32, isOutput=False)
    v_d = nc.declare_dram_parameter("v", [L, D], f32, isOutput=False)
    res_d = nc.declare_dram_parameter("residual", [L, DM], f32, isOutput=False)
    fcw_d = nc.declare_dram_parameter("fc_w", [DM, D], f32, isOutput=False)
    fcb_d = nc.declare_dram_parameter("fc_b", [DM], f32, isOutput=False)
    gam_d = nc.declare_dram_parameter("ln_gamma", [DM], f32, isOutput=False)
    bet_d = nc.declare_dram_parameter("ln_beta", [DM], f32, isOutput=False)
    out_d = nc.declare_dram_parameter("out", [L, DM], f32, isOutput=True)

    denom_s = nc.dram_tensor("denom_scratch", [L], f32)
    recip_s = nc.dram_tensor("recip_scratch", [L], f32)

    with tile.TileContext(nc) as tc:
        with (
            tc.tile_pool(name="raw", bufs=2) as raw_pool,
            tc.tile_pool(name="persist", bufs=1) as persist,
            tc.tile_pool(name="stage", bufs=2, space="PSUM") as stage_pool,
            tc.tile_pool(name="pv", bufs=2, space="PSUM") as pv_pool,
            tc.tile_pool(name="fc", bufs=1, space="PSUM") as fc_pool,
            tc.tile_pool(name="et", bufs=3) as et_pool,
            tc.tile_pool(name="resid", bufs=3) as res_pool,
            tc.tile_pool(name="x", bufs=3) as x_pool,
            tc.tile_pool(name="outs", bufs=3) as out_pool,
            tc.tile_pool(name="norm", bufs=2) as norm_pool,
            tc.tile_pool(name="small", bufs=4) as small_pool,
        ):
            identity = persist.tile([128, 128], f32)
            make_identity(nc, identity)
            eps_t = persist.tile([128, 1], f32, tag="eps")
            nc.vector.memset(eps_t, LN_EPS)

            # ---- load + transpose q, k -> qT, kT [64, 16, 128] ----
            qT = persist.tile([64, NTILES, 128], f32, tag="qT")
            kT = persist.tile([64, NTILES, 128], f32, tag="kT")
            for name, src, dstT in (("q", q_d, qT), ("k", k_d, kT)):
                raw = raw_pool.tile([128, NTILES, D], f32, tag="raw")
                nc.sync.dma_start(
                    out=raw, in_=src.ap().rearrange("(t p) d -> p t d", p=128)
                )
                # pair-transpose: 2 adjacent tiles per PE transpose, 4 per bank
                # tile index = grp*8 + pair*2 + par
                d4 = dstT.rearrange("d (grp pair par) c -> d grp pair par c",
                                    pair=4, par=2)
                for grp in range(NTILES // 8):
                    pt = stage_pool.tile([128, 512], f32, tag="stage")
                    for i in range(4):
                        nc.tensor.transpose(
                            pt[:, i * 128:(i + 1) * 128],
                            raw[:, (8 * grp + 2 * i): (8 * grp + 2 * i + 2), :],
                            identity,
                        )
                    ptv = pt.rearrange("p (four c) -> p four c", c=128)
                    # rows 0:63 = even tiles of each pair, 64:128 = odd tiles
                    nc.scalar.copy(out=d4[:, grp, :, 0, :], in_=ptv[0:64])
                    nc.scalar.copy(out=d4[:, grp, :, 1, :], in_=ptv[64:128])

            # ---- v with ones column: [128, 16, 65] ----
            v_sb = persist.tile([128, NTILES, D + 1], f32, tag="v")
            nc.sync.dma_start(
                out=v_sb[:, :, 0:D], in_=v_d.ap().rearrange("(t p) d -> p t d", p=128)
            )
            nc.vector.memset(v_sb[:, :, D:D + 1], 1.0)

            # ---- fc_wT augmented [65, 1024], row 64 = fc_b ----
            fcwT = persist.tile([65, DM], f32, tag="fcw")
            fraw = raw_pool.tile([128, DM // 128, D], f32, tag="raw")
            nc.sync.dma_start(
                out=fraw, in_=fcw_d.ap().rearrange("(t p) d -> p t d", p=128)
            )
            # fc tile index = pair*2 + par  (8 tiles of 128 rows)
            f4 = fcwT[0:64, :].rearrange("d (pair par c) -> d pair par c",
                                         par=2, c=128)
            pt = stage_pool.tile([128, 512], f32, tag="stage")
            for i in range(4):
                nc.tensor.transpose(
                    pt[:, i * 128:(i + 1) * 128],
                    fraw[:, 2 * i: 2 * i + 2, :],
                    identity,
                )
            ptv = pt.rearrange("p (four c) -> p four c", c=128)
            nc.scalar.copy(out=f4[:, :, 0, :], in_=ptv[0:64])
            nc.scalar.copy(out=f4[:, :, 1, :], in_=ptv[64:128])
            nc.sync.dma_start(
                out=fcwT[64:65, :],
                in_=bass.AP(tensor=fcb_d, offset=0, ap=[[0, 1], [1, DM]]),
            )

            if affine:
                gam_bc = persist.tile([128, DM], f32, tag="gam")
                bet_bc = persist.tile([128, DM], f32, tag="bet")
                nc.sync.dma_start(
                    out=gam_bc,
                    in_=bass.AP(tensor=gam_d, offset=0, ap=[[0, 128], [1, DM]]),
                )
                nc.sync.dma_start(
                    out=bet_bc,
                    in_=bass.AP(tensor=bet_d, offset=0, ap=[[0, 128], [1, DM]]),
                )

            kT2 = kT.rearrange("d t c -> d (t c)").rearrange(
                "d (g two c) -> d g two c", two=2, c=128
            )
            qT_flat = qT.rearrange("d t c -> d (t c)")

            # ---- main loop over q-slices ----
            for s in range(NSLICES):
                qs = qT_flat[:, s * 512:(s + 1) * 512]
                out_aug = pv_pool.tile([65, 512], f32, tag="pv")
                for g in range(NTILES // 2):
                    st = stage_pool.tile([128, 1024], f32, tag="stage")
                    nc.tensor.matmul(st[:, 0:512], kT2[:, g, 0, :], qs,
                                     start=True, stop=True)
                    nc.tensor.matmul(st[:, 512:1024], kT2[:, g, 1, :], qs,
                                     start=True, stop=True)
                    et = et_pool.tile([128, 1024], f32, tag="et")
                    nc.scalar.activation(
                        out=et, in_=st,
                        func=mybir.ActivationFunctionType.Exp, scale=SCALE,
                    )
                    nc.tensor.matmul(out_aug, v_sb[:, 2 * g, :], et[:, 0:512],
                                     start=(g == 0), stop=False)
                    nc.tensor.matmul(out_aug, v_sb[:, 2 * g + 1, :],
                                     et[:, 512:1024],
                                     start=False, stop=(g == NTILES // 2 - 1))

                # softmax denominator -> reciprocal in per-partition layout
                drow = small_pool.tile([1, 512], f32, tag="drow")
                nc.vector.tensor_copy(drow, out_aug[64:65, :])
                nc.sync.dma_start(
                    out=bass.AP(tensor=denom_s, offset=s * 512, ap=[[1, 512]]),
                    in_=drow,
                )
                dT = small_pool.tile([128, 4], f32, tag="dT")
                nc.sync.dma_start(
                    out=dT,
                    in_=bass.AP(tensor=denom_s, offset=s * 512,
                                ap=[[1, 128], [128, 4]]),
                )
                rT = small_pool.tile([128, 4], f32, tag="rT")
                nc.vector.reciprocal(rT, dT)
                nc.sync.dma_start(
                    out=bass.AP(tensor=recip_s, offset=s * 512,
                                ap=[[1, 128], [128, 4]]),
                    in_=rT,
                )
                rbc = norm_pool.tile([65, 512], f32, tag="rbc")
                nc.sync.dma_start(
                    out=rbc,
                    in_=bass.AP(tensor=recip_s, offset=s * 512,
                                ap=[[0, 65], [1, 512]]),
                )
                # normalize + evacuate PSUM in one op; row 64 -> exactly 1.0
                outT = norm_pool.tile([65, 512], f32, tag="outT")
                nc.vector.tensor_mul(outT, out_aug, rbc)

                for pi in range(4):
                    t = s * 4 + pi
                    fc_ps = fc_pool.tile([128, DM], f32, tag="fc")
                    lhs = outT[:, pi * 128:(pi + 1) * 128]
                    nc.tensor.matmul(fc_ps[:, 0:512], lhs, fcwT[:, 0:512],
                                     start=True, stop=True)
                    nc.tensor.matmul(fc_ps[:, 512:1024], lhs, fcwT[:, 512:1024],
                                     start=True, stop=True)
                    res_t = res_pool.tile([128, DM], f32, tag="res")
                    nc.sync.dma_start(
                        out=res_t, in_=res_d[t * 128:(t + 1) * 128, :]
                    )
                    x_t = x_pool.tile([128, DM], f32, tag="x")
                    nc.vector.tensor_add(x_t, fc_ps, res_t)
                    stats = small_pool.tile([128, 2, 6], f32, tag="stats")
                    nc.vector.bn_stats(out=stats[:, 0, :], in_=x_t[:, 0:512])
                    nc.vector.bn_stats(out=stats[:, 1, :], in_=x_t[:, 512:1024])
                    mv = small_pool.tile([128, 2], f32, tag="mv")
                    nc.vector.bn_aggr(out=mv, in_=stats)
                    logv = small_pool.tile([128, 1], f32, tag="logv")
                    nc.scalar.activation(
                        out=logv, in_=mv[:, 1:2],
                        func=mybir.ActivationFunctionType.Ln, bias=eps_t,
                    )
                    rstd = small_pool.tile([128, 1], f32, tag="rstd")
                    nc.scalar.activation(
                        out=rstd, in_=logv,
                        func=mybir.ActivationFunctionType.Exp, scale=-0.5,
                    )
                    out_t = out_pool.tile([128, DM], f32, tag="out")
                    nc.gpsimd.tensor_scalar(
                        out=out_t, in0=x_t,
                        scalar1=mv[:, 0:1], scalar2=rstd,
                        op0=mybir.AluOpType.subtract, op1=mybir.AluOpType.mult,
                    )
                    if affine:
                        nc.vector.tensor_mul(out_t, out_t, gam_bc)
                        nc.vector.tensor_add(out_t, out_t, bet_bc)
                    nc.sync.dma_start(
                        out=out_d[t * 128:(t + 1) * 128, :], in_=out_t
                    )

    nc.finalize()
    return nc


LAST_RESULTS = None


def kernel(q, k, v, residual, fc_w, fc_b, ln_gamma, ln_beta):
    from concourse.bass_utils import run_bass_kernel_spmd

    global LAST_RESULTS
    affine = not (
        np.allclose(ln_gamma, 1.0) and np.allclose(ln_beta, 0.0)
    )
    key = ("v1", affine)
    if key not in _CACHE:
        _CACHE[key] = _build(affine)
    nc = _CACHE[key]

    q = np.ascontiguousarray(q, dtype=np.float32)
    k = np.ascontiguousarray(k, dtype=np.float32)
    v = np.ascontiguousarray(v, dtype=np.float32)
    residual = np.ascontiguousarray(residual, dtype=np.float32)
    fc_w = np.ascontiguousarray(fc_w, dtype=np.float32)
    fc_b = np.ascontiguousarray(fc_b, dtype=np.float32)
    ln_gamma = np.ascontiguousarray(ln_gamma, dtype=np.float32)
    ln_beta = np.ascontiguousarray(ln_beta, dtype=np.float32)

    in_maps = [
        {
            "q": q[b], "k": k[b], "v": v[b], "residual": residual[b],
            "fc_w": fc_w, "fc_b": fc_b,
            "ln_gamma": ln_gamma, "ln_beta": ln_beta,
        }
        for b in range(B)
    ]
    res = run_bass_kernel_spmd(nc, in_maps, core_ids=list(range(B)))
    LAST_RESULTS = res
    return np.stack([res.results[b]["out"] for b in range(B)], axis=0)


# revision 10
# speedup vs baseline: 2.3747x; 2.3747x over previous
"""Fused attention + FC + residual + LayerNorm for Trainium2, 8 NeuronCores.

Problem: B=8, L=2048, d_k=d_v=64, d_model=1024, fp32 I/O.
Sharding: pure data parallel - batch element b -> core b. No collectives.

Per-core pipeline (Tile-scheduled):
  1. PE pair-transposes build qT/kT [64, 2048] and fc_wT [65, 1024] in bf16
     (the PSUM->SBUF evacuation copies double as the f32->bf16 cast).
     fc_wT row 64 = fc_b.
  2. Per q-slice of 512 (4 slices), loop k-tile pairs (8 groups):
     S^T [128k, 2x512q] on PE (bf16 in, f32 PSUM) -> exp on ScalarE with the
     1/8 temperature folded into the activation's free affine scale, output
     cast to bf16 -> PV matmul accumulates [65, 512] f32 (row 64 = softmax
     denominator via a ones-column appended to V).
  3. Normalization: denominator row round-trips through DRAM into a
     per-partition layout, reciprocal on DVE, broadcast-DMA'd back, fused
     into the PSUM evacuation as one tensor_tensor multiply (bf16 out).
     Row 64 becomes exactly denom*recip = 1, which is the bias row the
     K=65-augmented FC matmul needs.
  4. FC matmul (bf16, K=65 incl. bias row) -> DVE adds residual from PSUM ->
     bn_stats/bn_aggr -> rsqrt batched per slice as Ln/Exp on ScalarE (same
     ACT table set as the attention exp - enforced by filtering the table
     registry so only natural_log_exp_and_others provides Exp/Ln/Copy) ->
     LN apply on ScalarE as Copy with per-partition scale/bias.
"""
import numpy as np

B = 8
L = 2048
D = 64
DM = 1024
NTILES = L // 128       # 16 q/k tiles of 128
NSLICES = L // 512      # 4 q-slices of 512
LN_EPS = 1e-5
SCALE = 0.125           # 1/sqrt(64)

_CACHE = {}
_TABLES_PATCHED = False


def _patch_act_tables():
    """Force every activation we use into one table set so the scheduler
    never needs a mid-kernel ACT_TABLE_LOAD switch (Exp <-> Ln)."""
    global _TABLES_PATCHED
    if _TABLES_PATCHED:
        return
    import concourse.bacc as bacc
    from concourse import mybir

    orig = bacc.get_activation_tables
    keep = "natural_log_exp_and_others"
    shared = {
        mybir.ActivationFunctionType.Exp,
        mybir.ActivationFunctionType.Ln,
        mybir.ActivationFunctionType.Copy,
        mybir.ActivationFunctionType.Identity,
        mybir.ActivationFunctionType.Square,
    }

    def patched(arch):
        tables = orig(arch)
        for name, fns in tables.items():
            if name != keep:
                fns.difference_update(shared)
        return tables

    bacc.get_activation_tables = patched
    _TABLES_PATCHED = True


def _build(affine: bool):
    import concourse.bacc as bacc
    import concourse.tile as tile
    from concourse import mybir
    import concourse.bass as bass
    from concourse.masks import make_identity

    _patch_act_tables()
    f32 = mybir.dt.float32
    bf16 = mybir.dt.bfloat16
    nc = bacc.Bacc("TRN2", target_bir_lowering=False, debug=False, num_devices=B)

    q_d = nc.declare_dram_parameter("q", [L, D], f32, isOutput=False)
    k_d = nc.declare_dram_parameter("k", [L, D], f32, isOutput=False)
    v_d = nc.declare_dram_parameter("v", [L, D], f32, isOutput=False)
    res_d = nc.declare_dram_parameter("residual", [L, DM], f32, isOutput=False)
    fcw_d = nc.declare_dram_parameter("fc_w", [DM, D], f32, isOutput=False)
    fcb_d = nc.declare_dram_parameter("fc_b", [DM], f32, isOutput=False)
    gam_d = nc.declare_dram_parameter("ln_gamma", [DM], f32, isOutput=False)
    bet_d = nc.declare_dram_parameter("ln_beta", [DM], f32, isOutput=False)
    out_d = nc.declare_dram_parameter("out", [L, DM], f32, isOutput=True)

    denom_s = nc.dram_tensor("denom_scratch", [L], f32)
    recip_s = nc.dram_tensor("recip_scratch", [L], f32)

    with tile.TileContext(nc) as tc:
        with (
            tc.tile_pool(name="raw", bufs=2) as raw_pool,
            tc.tile_pool(name="persist", bufs=1) as persist,
            tc.tile_pool(name="stage", bufs=2, space="PSUM") as stage_pool,
            tc.tile_pool(name="pv", bufs=2, space="PSUM") as pv_pool,
            tc.tile_pool(name="fc", bufs=1, space="PSUM") as fc_pool,
            tc.tile_pool(name="et", bufs=4) as et_pool,
            tc.tile_pool(name="resid", bufs=3) as res_pool,
            tc.tile_pool(name="x", bufs=6) as x_pool,
            tc.tile_pool(name="outs", bufs=3) as out_pool,
            tc.tile_pool(name="norm", bufs=2) as norm_pool,
            tc.tile_pool(name="small", bufs=4) as small_pool,
        ):
            identity = persist.tile([128, 128], f32)
            make_identity(nc, identity)
            eps_t = persist.tile([128, 1], f32, tag="eps")
            nc.vector.memset(eps_t, LN_EPS)

            # ---- load + transpose q, k -> qT, kT [64, 16, 128] bf16 ----
            qT = persist.tile([64, NTILES, 128], bf16, tag="qT")
            kT = persist.tile([64, NTILES, 128], bf16, tag="kT")
            for src, dstT in ((q_d, qT), (k_d, kT)):
                raw = raw_pool.tile([128, NTILES, D], f32, tag="raw")
                nc.sync.dma_start(
                    out=raw, in_=src.ap().rearrange("(t p) d -> p t d", p=128)
                )
                # pair-transpose: 2 adjacent 128x64 tiles per PE transpose
                # tile index = grp*8 + pair*2 + par
                d4 = dstT.rearrange("d (grp pair par) c -> d grp pair par c",
                                    pair=4, par=2)
                for grp in range(NTILES // 8):
                    pt = stage_pool.tile([128, 512], f32, tag="stage")
                    for i in range(4):
                        nc.tensor.transpose(
                            pt[:, i * 128:(i + 1) * 128],
                            raw[:, (8 * grp + 2 * i): (8 * grp + 2 * i + 2), :],
                            identity,
                        )
                    ptv = pt.rearrange("p (four c) -> p four c", c=128)
                    # rows 0:63 = even tiles of each pair, 64:128 = odd tiles
                    # (copy shifts partitions and casts f32 -> bf16)
                    nc.vector.tensor_copy(d4[:, grp, :, 0, :], ptv[0:64])
                    nc.vector.tensor_copy(d4[:, grp, :, 1, :], ptv[64:128])

            # ---- v with ones column: [128, 16, 65] bf16 ----
            vraw = raw_pool.tile([128, NTILES, D], f32, tag="raw")
            nc.sync.dma_start(
                out=vraw, in_=v_d.ap().rearrange("(t p) d -> p t d", p=128)
            )
            v_sb = persist.tile([128, NTILES, D + 1], bf16, tag="v")
            nc.vector.tensor_copy(v_sb[:, :, 0:D], vraw)
            nc.vector.memset(v_sb[:, :, D:D + 1], 1.0)

            # ---- fc_wT augmented [65, 1024] bf16, row 64 = fc_b ----
            fcwT = persist.tile([65, DM], bf16, tag="fcw")
            fraw = raw_pool.tile([128, DM // 128, D], f32, tag="raw")
            nc.sync.dma_start(
                out=fraw, in_=fcw_d.ap().rearrange("(t p) d -> p t d", p=128)
            )
            # fc tile index = pair*2 + par  (8 tiles of 128 rows)
            f4 = fcwT[0:64, :].rearrange("d (pair par c) -> d pair par c",
                                         par=2, c=128)
            pt = stage_pool.tile([128, 512], f32, tag="stage")
            for i in range(4):
                nc.tensor.transpose(
                    pt[:, i * 128:(i + 1) * 128],
                    fraw[:, 2 * i: 2 * i + 2, :],
                    identity,
                )
            ptv = pt.rearrange("p (four c) -> p four c", c=128)
            nc.vector.tensor_copy(f4[:, :, 0, :], ptv[0:64])
            nc.vector.tensor_copy(f4[:, :, 1, :], ptv[64:128])
            fcb_t = small_pool.tile([1, DM], f32, tag="fcb")
            nc.sync.dma_start(
                out=fcb_t,
                in_=bass.AP(tensor=fcb_d, offset=0, ap=[[0, 1], [1, DM]]),
            )
            nc.vector.tensor_copy(fcwT[64:65, :], fcb_t)

            if affine:
                gam_bc = persist.tile([128, DM], f32, tag="gam")
                bet_bc = persist.tile([128, DM], f32, tag="bet")
                nc.sync.dma_start(
                    out=gam_bc,
                    in_=bass.AP(tensor=gam_d, offset=0, ap=[[0, 128], [1, DM]]),
                )
                nc.sync.dma_start(
                    out=bet_bc,
                    in_=bass.AP(tensor=bet_d, offset=0, ap=[[0, 128], [1, DM]]),
                )

            kT2 = kT.rearrange("d t c -> d (t c)").rearrange(
                "d (g two c) -> d g two c", two=2, c=128
            )
            qT_flat = qT.rearrange("d t c -> d (t c)")

            # ---- main loop over q-slices ----
            for s in range(NSLICES):
                qs = qT_flat[:, s * 512:(s + 1) * 512]
                out_aug = pv_pool.tile([65, 512], f32, tag="pv")
                for g in range(NTILES // 2):
                    st = stage_pool.tile([128, 1024], f32, tag="stage")
                    nc.tensor.matmul(st[:, 0:512], kT2[:, g, 0, :], qs,
                                     start=True, stop=True)
                    nc.tensor.matmul(st[:, 512:1024], kT2[:, g, 1, :], qs,
                                     start=True, stop=True)
                    et = et_pool.tile([128, 1024], bf16, tag="et")
                    nc.scalar.activation(
                        out=et, in_=st,
                        func=mybir.ActivationFunctionType.Exp, scale=SCALE,
                    )
                    nc.tensor.matmul(out_aug, v_sb[:, 2 * g, :], et[:, 0:512],
                                     start=(g == 0), stop=False)
                    nc.tensor.matmul(out_aug, v_sb[:, 2 * g + 1, :],
                                     et[:, 512:1024],
                                     start=False, stop=(g == NTILES // 2 - 1))

                # softmax denominator -> reciprocal in per-partition layout
                drow = small_pool.tile([1, 512], f32, tag="drow")
                nc.vector.tensor_copy(drow, out_aug[64:65, :])
                nc.sync.dma_start(
                    out=bass.AP(tensor=denom_s, offset=s * 512, ap=[[1, 512]]),
                    in_=drow,
                )
                dT = small_pool.tile([128, 4], f32, tag="dT")
                nc.sync.dma_start(
                    out=dT,
                    in_=bass.AP(tensor=denom_s, offset=s * 512,
                                ap=[[1, 128], [128, 4]]),
                )
                rT = small_pool.tile([128, 4], f32, tag="rT")
                nc.vector.reciprocal(rT, dT)
                nc.sync.dma_start(
                    out=bass.AP(tensor=recip_s, offset=s * 512,
                                ap=[[1, 128], [128, 4]]),
                    in_=rT,
                )
                rbc = norm_pool.tile([65, 512], f32, tag="rbc")
                nc.sync.dma_start(
                    out=rbc,
                    in_=bass.AP(tensor=recip_s, offset=s * 512,
                                ap=[[0, 65], [1, 512]]),
                )
                # normalize + evacuate PSUM + cast bf16; row 64 -> exactly 1.0
                outT = norm_pool.tile([65, 512], bf16, tag="outT")
                nc.vector.tensor_mul(outT, out_aug, rbc)

                mv_all = small_pool.tile([128, 4, 2], f32, tag="mv")
                x_ts = []
                for pi in range(4):
                    t = s * 4 + pi
                    fc_ps = fc_pool.tile([128, DM], f32, tag="fc")
                    lhs = outT[:, pi * 128:(pi + 1) * 128]
                    nc.tensor.matmul(fc_ps[:, 0:512], lhs, fcwT[:, 0:512],
                                     start=True, stop=True)
                    nc.tensor.matmul(fc_ps[:, 512:1024], lhs, fcwT[:, 512:1024],
                                     start=True, stop=True)
                    res_t = res_pool.tile([128, DM], f32, tag="res")
                    nc.sync.dma_start(
                        out=res_t, in_=res_d[t * 128:(t + 1) * 128, :]
                    )
                    x_t = x_pool.tile([128, DM], f32, tag="x")
                    nc.vector.tensor_add(x_t, fc_ps, res_t)
                    x_ts.append(x_t)
                    stats = small_pool.tile([128, 2, 6], f32, tag="stats")
                    nc.vector.bn_stats(out=stats[:, 0, :], in_=x_t[:, 0:512])
                    nc.vector.bn_stats(out=stats[:, 1, :], in_=x_t[:, 512:1024])
                    nc.vector.bn_aggr(out=mv_all[:, pi, :], in_=stats)

                # batched rsqrt for the slice: rstd = exp(-0.5*ln(var+eps))
                rstd4 = small_pool.tile([128, 4], f32, tag="rstd")
                nc.scalar.activation(
                    out=rstd4, in_=mv_all[:, :, 1],
                    func=mybir.ActivationFunctionType.Ln, bias=eps_t,
                )
                nc.scalar.activation(
                    out=rstd4, in_=rstd4,
                    func=mybir.ActivationFunctionType.Exp, scale=-0.5,
                )
                nm4 = small_pool.tile([128, 4], f32, tag="nm")
                nc.vector.tensor_tensor(
                    out=nm4, in0=mv_all[:, :, 0], in1=rstd4,
                    op=mybir.AluOpType.mult,
                )
                nc.vector.tensor_scalar_mul(out=nm4, in0=nm4, scalar1=-1.0)

                for pi in range(4):
                    t = s * 4 + pi
                    out_t = out_pool.tile([128, DM], f32, tag="out")
                    # (x - mu) * rstd  ==  x*rstd + (-mu*rstd)
                    nc.scalar.activation(
                        out=out_t, in_=x_ts[pi],
                        func=mybir.ActivationFunctionType.Identity,
                        bias=nm4[:, pi:pi + 1], scale=rstd4[:, pi:pi + 1],
                    )
                    if affine:
                        nc.vector.tensor_mul(out_t, out_t, gam_bc)
                        nc.vector.tensor_add(out_t, out_t, bet_bc)
                    nc.sync.dma_start(
                        out=out_d[t * 128:(t + 1) * 128, :], in_=out_t
                    )

    nc.finalize()
    return nc


LAST_RESULTS = None


def kernel(q, k, v, residual, fc_w, fc_b, ln_gamma, ln_beta):
    from concourse.bass_utils import run_bass_kernel_spmd

    global LAST_RESULTS
    affine = not (
        np.allclose(ln_gamma, 1.0) and np.allclose(ln_beta, 0.0)
    )
    key = ("v2", affine)
    if key not in _CACHE:
        _CACHE[key] = _build(affine)
    nc = _CACHE[key]

    q = np.ascontiguousarray(q, dtype=np.float32)
    k = np.ascontiguousarray(k, dtype=np.float32)
    v = np.ascontiguousarray(v, dtype=np.float32)
    residual = np.ascontiguousarray(residual, dtype=np.float32)
    fc_w = np.ascontiguousarray(fc_w, dtype=np.float32)
    fc_b = np.ascontiguousarray(fc_b, dtype=np.float32)
    ln_gamma = np.ascontiguousarray(ln_gamma, dtype=np.float32)
    ln_beta = np.ascontiguousarray(ln_beta, dtype=np.float32)

    in_maps = [
        {
            "q": q[b], "k": k[b], "v": v[b], "residual": residual[b],
            "fc_w": fc_w, "fc_b": fc_b,
            "ln_gamma": ln_gamma, "ln_beta": ln_beta,
        }
        for b in range(B)
    ]
    res = run_bass_kernel_spmd(nc, in_maps, core_ids=list(range(B)))
    LAST_RESULTS = res
    return np.stack([res.results[b]["out"] for b in range(B)], axis=0)


# revision 11
# speedup vs baseline: 2.4200x; 1.0191x over previous
"""Fused attention + FC + residual + LayerNorm for Trainium2, 8 NeuronCores.

Problem: B=8, L=2048, d_k=d_v=64, d_model=1024, fp32 I/O.
Sharding: pure data parallel - batch element b -> core b. No collectives.

Per-core pipeline (Tile-scheduled, software-pipelined one q-slice deep so the
strict-FIFO engine queues never stall on cross-slice dependencies):

  iter s:  attention(s)  ->  epilogue(s-1)  ->  denominator-dance(s)

  attention(s): per k-tile pair, S^T [128k, 2x512q] on PE (bf16 operands,
    f32 PSUM) -> exp on ScalarE with the 1/sqrt(d) temperature folded into
    the activation's free affine scale, bf16 out -> PV matmul accumulates
    [65, 512] f32 where row 64 is the softmax denominator (ones-column
    appended to V).
  dance(s): denominator row -> SBUF, transposed to a per-partition [128, 4]
    layout with four tiny K=1 PE matmuls, reciprocal on DVE, then DMA to
    DRAM + broadcast-DMA back as [65, 512]; the PSUM evacuation fuses the
    normalize as one tensor_tensor multiply (bf16 out). Row 64 becomes
    exactly denom*recip = 1, the bias row the K=65-augmented FC needs.
  epilogue(s): FC matmul (bf16, K=65 incl. fc_b row) -> DVE adds residual
    from PSUM -> bn_stats/bn_aggr -> rsqrt batched per slice as Ln/Exp on
    ScalarE (kept in the same ACT table set as the attention exp by
    filtering the table registry) -> LN apply on DVE tensor_scalar
    (fp32 SBUF 2x mode) -> store.

  qT/kT [64, 2048] and fc_wT [65, 1024] (row 64 = fc_b) are built by PE
  transposes from contiguous loads; the PSUM evacuation copies double as
  the f32->bf16 cast.
"""
import numpy as np

B = 8
L = 2048
D = 64
DM = 1024
NTILES = L // 128       # 16 q/k tiles of 128
NSLICES = L // 512      # 4 q-slices of 512
LN_EPS = 1e-5
SCALE = 0.125           # 1/sqrt(64)

_CACHE = {}
_TABLES_PATCHED = False


def _patch_act_tables():
    """Force every activation we use into one table set so the scheduler
    never needs a mid-kernel ACT_TABLE_LOAD switch (Exp <-> Ln)."""
    global _TABLES_PATCHED
    if _TABLES_PATCHED:
        return
    import concourse.bacc as bacc
    from concourse import mybir

    orig = bacc.get_activation_tables
    keep = "natural_log_exp_and_others"
    shared = {
        mybir.ActivationFunctionType.Exp,
        mybir.ActivationFunctionType.Ln,
        mybir.ActivationFunctionType.Copy,
        mybir.ActivationFunctionType.Identity,
        mybir.ActivationFunctionType.Square,
    }

    def patched(arch):
        tables = orig(arch)
        for name, fns in tables.items():
            if name != keep:
                fns.difference_update(shared)
        return tables

    bacc.get_activation_tables = patched
    _TABLES_PATCHED = True


def _build(affine: bool):
    import concourse.bacc as bacc
    import concourse.tile as tile
    from concourse import mybir
    import concourse.bass as bass
    from concourse.masks import make_identity

    _patch_act_tables()
    f32 = mybir.dt.float32
    bf16 = mybir.dt.bfloat16
    nc = bacc.Bacc("TRN2", target_bir_lowering=False, debug=False, num_devices=B)

    q_d = nc.declare_dram_parameter("q", [L, D], f32, isOutput=False)
    k_d = nc.declare_dram_parameter("k", [L, D], f32, isOutput=False)
    v_d = nc.declare_dram_parameter("v", [L, D], f32, isOutput=False)
    res_d = nc.declare_dram_parameter("residual", [L, DM], f32, isOutput=False)
    fcw_d = nc.declare_dram_parameter("fc_w", [DM, D], f32, isOutput=False)
    fcb_d = nc.declare_dram_parameter("fc_b", [DM], f32, isOutput=False)
    gam_d = nc.declare_dram_parameter("ln_gamma", [DM], f32, isOutput=False)
    bet_d = nc.declare_dram_parameter("ln_beta", [DM], f32, isOutput=False)
    out_d = nc.declare_dram_parameter("out", [L, DM], f32, isOutput=True)

    recip_s = nc.dram_tensor("recip_scratch", [L], f32)

    with tile.TileContext(nc) as tc:
        with (
            tc.tile_pool(name="raw", bufs=2) as raw_pool,
            tc.tile_pool(name="persist", bufs=1) as persist,
            tc.tile_pool(name="stage", bufs=2, space="PSUM") as stage_pool,
            tc.tile_pool(name="pv", bufs=2, space="PSUM") as pv_pool,
            tc.tile_pool(name="fc", bufs=1, space="PSUM") as fc_pool,
            tc.tile_pool(name="et", bufs=4) as et_pool,
            tc.tile_pool(name="resid", bufs=4) as res_pool,
            tc.tile_pool(name="x", bufs=8) as x_pool,
            tc.tile_pool(name="outs", bufs=4) as out_pool,
            tc.tile_pool(name="norm", bufs=2) as norm_pool,
            tc.tile_pool(name="small", bufs=4) as small_pool,
        ):
            identity = persist.tile([128, 128], f32)
            make_identity(nc, identity)
            eps_t = persist.tile([128, 1], f32, tag="eps")
            nc.vector.memset(eps_t, LN_EPS)
            one_c = persist.tile([1, 1], f32, tag="onec")
            nc.vector.memset(one_c, 1.0)

            # ---- contiguous loads + PE transposes -> qT/kT [64, 2048] bf16
            # raw partition p holds rows 16p..16p+15; transpose slice j has
            # rows (2j, 2j+1); qT column = 16c + 8g + 2j + r
            qT = persist.tile([64, L], bf16, tag="qT")
            kT = persist.tile([64, L], bf16, tag="kT")
            for src, dstT in ((q_d, qT), (k_d, kT)):
                raw = raw_pool.tile([128, NTILES * D], f32, tag="raw")
                nc.sync.dma_start(
                    out=raw,
                    in_=src.ap().rearrange("(p r) d -> p (r d)", p=128),
                )
                d5 = dstT.rearrange("d (c g j r) -> d c g j r", g=2, j=4, r=2)
                for g in range(2):
                    pt = stage_pool.tile([128, 512], f32, tag="stage")
                    for j in range(4):
                        nc.tensor.transpose(
                            pt[:, j * 128:(j + 1) * 128],
                            raw[:, (4 * g + j) * 128:(4 * g + j + 1) * 128],
                            identity,
                        )
                    ptv = pt.rearrange("p (j c) -> p j c", c=128)
                    nc.scalar.copy(
                        out=d5[:, :, g, :, 0].rearrange("d c j -> d j c"),
                        in_=ptv[0:64],
                    )
                    nc.scalar.copy(
                        out=d5[:, :, g, :, 1].rearrange("d c j -> d j c"),
                        in_=ptv[64:128],
                    )

            # ---- v with ones column: [128, 16, 65] bf16 (tile-major load)
            vraw = raw_pool.tile([128, NTILES, D], f32, tag="raw")
            nc.sync.dma_start(
                out=vraw, in_=v_d.ap().rearrange("(t p) d -> p t d", p=128)
            )
            v_sb = persist.tile([128, NTILES, D + 1], bf16, tag="v")
            nc.scalar.copy(v_sb[:, :, 0:D], vraw)
            nc.vector.memset(v_sb[:, :, D:D + 1], 1.0)

            # ---- fc_wT augmented [65, 1024] bf16, row 64 = fc_b ----
            # contiguous: raw partition p = 8 rows; fcwT col = 8c + 2j + r
            fcwT = persist.tile([65, DM], bf16, tag="fcw")
            fraw = raw_pool.tile([128, (DM // 128) * D], f32, tag="raw")
            nc.sync.dma_start(
                out=fraw, in_=fcw_d.ap().rearrange("(p r) d -> p (r d)", p=128)
            )
            f5 = fcwT[0:64, :].rearrange("d (c j r) -> d c j r", j=4, r=2)
            pt = stage_pool.tile([128, 512], f32, tag="stage")
            for j in range(4):
                nc.tensor.transpose(
                    pt[:, j * 128:(j + 1) * 128],
                    fraw[:, j * 128:(j + 1) * 128],
                    identity,
                )
            ptv = pt.rearrange("p (j c) -> p j c", c=128)
            nc.scalar.copy(out=f5[:, :, :, 0].rearrange("d c j -> d j c"),
                           in_=ptv[0:64])
            nc.scalar.copy(out=f5[:, :, :, 1].rearrange("d c j -> d j c"),
                           in_=ptv[64:128])
            fcb_t = small_pool.tile([1, DM], f32, tag="fcb")
            nc.sync.dma_start(
                out=fcb_t,
                in_=bass.AP(tensor=fcb_d, offset=0, ap=[[0, 1], [1, DM]]),
            )
            nc.vector.tensor_copy(fcwT[64:65, :], fcb_t)

            if affine:
                gam_bc = persist.tile([128, DM], f32, tag="gam")
                bet_bc = persist.tile([128, DM], f32, tag="bet")
                nc.sync.dma_start(
                    out=gam_bc,
                    in_=bass.AP(tensor=gam_d, offset=0, ap=[[0, 128], [1, DM]]),
                )
                nc.sync.dma_start(
                    out=bet_bc,
                    in_=bass.AP(tensor=bet_d, offset=0, ap=[[0, 128], [1, DM]]),
                )

            kT2 = kT.rearrange("d (g two c) -> d g two c", two=2, c=128)

            state = {}

            def attention(s):
                qs = qT[:, s * 512:(s + 1) * 512]
                out_aug = pv_pool.tile([65, 512], f32, tag="pv")
                for g in range(NTILES // 2):
                    st = stage_pool.tile([128, 1024], f32, tag="stage")
                    nc.tensor.matmul(st[:, 0:512], kT2[:, g, 0, :], qs,
                                     start=True, stop=True)
                    nc.tensor.matmul(st[:, 512:1024], kT2[:, g, 1, :], qs,
                                     start=True, stop=True)
                    et = et_pool.tile([128, 1024], bf16, tag="et")
                    nc.scalar.activation(
                        out=et, in_=st,
                        func=mybir.ActivationFunctionType.Exp, scale=SCALE,
                    )
                    nc.tensor.matmul(out_aug, v_sb[:, 2 * g, :], et[:, 0:512],
                                     start=(g == 0), stop=False)
                    nc.tensor.matmul(out_aug, v_sb[:, 2 * g + 1, :],
                                     et[:, 512:1024],
                                     start=False, stop=(g == NTILES // 2 - 1))
                return out_aug

            def dance(s, out_aug):
                # denom row -> per-partition [128, 4] via 4 tiny PE matmuls
                drow = small_pool.tile([1, 512], f32, tag="drow")
                nc.scalar.copy(drow, out_aug[64:65, :])
                dT = stage_pool.tile([128, 4], f32, tag="stage")
                for t in range(4):
                    nc.tensor.matmul(dT[:, t:t + 1],
                                     drow[:, t * 128:(t + 1) * 128], one_c,
                                     start=True, stop=True)
                rT = small_pool.tile([128, 4], f32, tag="rT")
                nc.vector.reciprocal(rT, dT)
                nc.sync.dma_start(
                    out=bass.AP(tensor=recip_s, offset=s * 512,
                                ap=[[1, 128], [128, 4]]),
                    in_=rT,
                )
                rbc = norm_pool.tile([65, 512], f32, tag="rbc")
                nc.sync.dma_start(
                    out=rbc,
                    in_=bass.AP(tensor=recip_s, offset=s * 512,
                                ap=[[0, 65], [1, 512]]),
                )
                # normalize + evacuate PSUM + cast bf16; row 64 -> exactly 1.0
                outT = norm_pool.tile([65, 512], bf16, tag="outT")
                nc.vector.tensor_mul(outT, out_aug, rbc)
                state[s] = {"outT": outT}

            def epilogue(s):
                outT = state[s]["outT"]
                mv_all = small_pool.tile([128, 4, 2], f32, tag="mv")
                x_ts = []
                for pi in range(4):
                    t = s * 4 + pi
                    fc_ps = fc_pool.tile([128, DM], f32, tag="fc")
                    lhs = outT[:, pi * 128:(pi + 1) * 128]
                    nc.tensor.matmul(fc_ps[:, 0:512], lhs, fcwT[:, 0:512],
                                     start=True, stop=True)
                    nc.tensor.matmul(fc_ps[:, 512:1024], lhs,
                                     fcwT[:, 512:1024],
                                     start=True, stop=True)
                    res_t = res_pool.tile([128, DM], f32, tag="res")
                    nc.sync.dma_start(
                        out=res_t, in_=res_d[t * 128:(t + 1) * 128, :]
                    )
                    x_t = x_pool.tile([128, DM], f32, tag="x")
                    nc.vector.tensor_add(x_t, fc_ps, res_t)
                    x_ts.append(x_t)
                    stats = small_pool.tile([128, 2, 6], f32, tag="stats")
                    nc.vector.bn_stats(out=stats[:, 0, :], in_=x_t[:, 0:512])
                    nc.vector.bn_stats(out=stats[:, 1, :],
                                       in_=x_t[:, 512:1024])
                    nc.vector.bn_aggr(out=mv_all[:, pi, :], in_=stats)

                # batched rsqrt: rstd = exp(-0.5*ln(var+eps))
                rstd4 = small_pool.tile([128, 4], f32, tag="rstd")
                nc.scalar.activation(
                    out=rstd4, in_=mv_all[:, :, 1],
                    func=mybir.ActivationFunctionType.Ln, bias=eps_t,
                )
                nc.scalar.activation(
                    out=rstd4, in_=rstd4,
                    func=mybir.ActivationFunctionType.Exp, scale=-0.5,
                )

                for pi in range(4):
                    t = s * 4 + pi
                    out_t = out_pool.tile([128, DM], f32, tag="out")
                    nc.vector.tensor_scalar(
                        out=out_t, in0=x_ts[pi],
                        scalar1=mv_all[:, pi, 0:1],
                        scalar2=rstd4[:, pi:pi + 1],
                        op0=mybir.AluOpType.subtract,
                        op1=mybir.AluOpType.mult,
                    )
                    if affine:
                        nc.vector.tensor_mul(out_t, out_t, gam_bc)
                        nc.vector.tensor_add(out_t, out_t, bet_bc)
                    nc.sync.dma_start(
                        out=out_d[t * 128:(t + 1) * 128, :], in_=out_t
                    )
                del state[s]

            # software pipeline: epilogue runs one slice behind attention so
            # the strict-FIFO PE queue never waits on the denominator dance
            for s in range(NSLICES + 1):
                if s < NSLICES:
                    oa = attention(s)
                if s >= 1:
                    epilogue(s - 1)
                if s < NSLICES:
                    dance(s, oa)

    nc.finalize()
    return nc


LAST_RESULTS = None


def kernel(q, k, v, residual, fc_w, fc_b, ln_gamma, ln_beta):
    from concourse.bass_utils import run_bass_kernel_spmd

    global LAST_RESULTS
    affine = not (
        np.allclose(ln_gamma, 1.0) and np.allclose(ln_beta, 0.0)
    )
    key = ("v3", affine)
    if key not in _CACHE:
        _CACHE[key] = _build(affine)
    nc = _CACHE[key]

    q = np.ascontiguousarray(q, dtype=np.float32)
    k = np.ascontiguousarray(k, dtype=np.float32)
    v = np.ascontiguousarray(v, dtype=np.float32)
    residual = np.ascontiguousarray(residual, dtype=np.float32)
    fc_w = np.ascontiguousarray(fc_w, dtype=np.float32)
    fc_b = np.ascontiguousarray(fc_b, dtype=np.float32)
    ln_gamma = np.ascontiguousarray(ln_gamma, dtype=np.float32)
    ln_beta = np.ascontiguousarray(ln_beta, dtype=np.float32)

    in_maps = [
        {
            "q": q[b], "k": k[b], "v": v[b], "residual": residual[b],
            "fc_w": fc_w, "fc_b": fc_b,
            "ln_gamma": ln_gamma, "ln_beta": ln_beta,
        }
        for b in range(B)
    ]
    res = run_bass_kernel_spmd(nc, in_maps, core_ids=list(range(B)))
    LAST_RESULTS = res
    return np.stack([res.results[b]["out"] for b in range(B)], axis=0)


# revision 12
# speedup vs baseline: 2.5312x; 1.0460x over previous
"""Fused attention + FC + residual + LayerNorm for Trainium2, 8 NeuronCores.

Problem: B=8, L=2048, d_k=d_v=64, d_model=1024, fp32 I/O.
Sharding: pure data parallel - batch element b -> core b. No collectives.

Per-core pipeline (Tile-scheduled, software-pipelined one q-slice deep so the
strict-FIFO engine queues never stall on cross-slice dependencies):

  iter s:  attention(s)  ->  epilogue(s-1)  ->  denominator-dance(s)

  attention(s): per k-tile pair, S^T [128k, 2x512q] on PE (bf16 operands,
    f32 PSUM) -> exp on ScalarE with the 1/sqrt(d) temperature folded into
    the activation's free affine scale, bf16 out -> PV matmul accumulates
    [65, 512] f32 where row 64 is the softmax denominator (ones-column
    appended to V).
  dance(s): denominator row -> SBUF, transposed to a per-partition [128, 4]
    layout with four tiny K=1 PE matmuls, reciprocal on DVE, then DMA to
    DRAM + broadcast-DMA back as [65, 512]; the PSUM evacuation fuses the
    normalize as one tensor_tensor multiply (bf16 out). Row 64 becomes
    exactly denom*recip = 1, the bias row the K=65-augmented FC needs.
  epilogue(s): FC matmul (bf16, K=65 incl. fc_b row) -> DVE adds residual
    from PSUM -> bn_stats/bn_aggr -> rsqrt batched per slice as Ln/Exp on
    ScalarE (kept in the same ACT table set as the attention exp by
    filtering the table registry) -> LN apply on DVE tensor_scalar
    (fp32 SBUF 2x mode) -> store.

  qT/kT [64, 2048] and fc_wT [65, 1024] (row 64 = fc_b) are built by PE
  transposes from contiguous loads; the PSUM evacuation copies double as
  the f32->bf16 cast.
"""
import numpy as np

B = 8
L = 2048
D = 64
DM = 1024
NTILES = L // 128       # 16 q/k tiles of 128
NSLICES = L // 512      # 4 q-slices of 512
LN_EPS = 1e-5
SCALE = 0.125           # 1/sqrt(64)

_CACHE = {}
_TABLES_PATCHED = False


def _patch_act_tables():
    """Force every activation we use into one table set so the scheduler
    never needs a mid-kernel ACT_TABLE_LOAD switch (Exp <-> Ln)."""
    global _TABLES_PATCHED
    if _TABLES_PATCHED:
        return
    import concourse.bacc as bacc
    from concourse import mybir

    orig = bacc.get_activation_tables
    keep = "natural_log_exp_and_others"
    shared = {
        mybir.ActivationFunctionType.Exp,
        mybir.ActivationFunctionType.Ln,
        mybir.ActivationFunctionType.Copy,
        mybir.ActivationFunctionType.Identity,
        mybir.ActivationFunctionType.Square,
    }

    def patched(arch):
        tables = orig(arch)
        for name, fns in tables.items():
            if name != keep:
                fns.difference_update(shared)
        return tables

    bacc.get_activation_tables = patched
    _TABLES_PATCHED = True


def _build(affine: bool):
    import concourse.bacc as bacc
    import concourse.tile as tile
    from concourse import mybir
    import concourse.bass as bass
    from concourse.masks import make_identity

    _patch_act_tables()
    f32 = mybir.dt.float32
    bf16 = mybir.dt.bfloat16
    nc = bacc.Bacc("TRN2", target_bir_lowering=False, debug=False, num_devices=B)

    q_d = nc.declare_dram_parameter("q", [L, D], f32, isOutput=False)
    k_d = nc.declare_dram_parameter("k", [L, D], f32, isOutput=False)
    v_d = nc.declare_dram_parameter("v", [L, D], f32, isOutput=False)
    res_d = nc.declare_dram_parameter("residual", [L, DM], f32, isOutput=False)
    fcw_d = nc.declare_dram_parameter("fc_w", [DM, D], f32, isOutput=False)
    fcb_d = nc.declare_dram_parameter("fc_b", [DM], f32, isOutput=False)
    gam_d = nc.declare_dram_parameter("ln_gamma", [DM], f32, isOutput=False)
    bet_d = nc.declare_dram_parameter("ln_beta", [DM], f32, isOutput=False)
    out_d = nc.declare_dram_parameter("out", [L, DM], f32, isOutput=True)

    recip_s = nc.dram_tensor("recip_scratch", [L], f32)

    with tile.TileContext(nc) as tc:
        with (
            tc.tile_pool(name="raw", bufs=2) as raw_pool,
            tc.tile_pool(name="persist", bufs=1) as persist,
            tc.tile_pool(name="stage", bufs=2, space="PSUM") as stage_pool,
            tc.tile_pool(name="pv", bufs=2, space="PSUM") as pv_pool,
            tc.tile_pool(name="fc", bufs=1, space="PSUM") as fc_pool,
            tc.tile_pool(name="et", bufs=6) as et_pool,
            tc.tile_pool(name="resid", bufs=6) as res_pool,
            tc.tile_pool(name="x", bufs=8) as x_pool,
            tc.tile_pool(name="outs", bufs=4) as out_pool,
            tc.tile_pool(name="norm", bufs=2) as norm_pool,
            tc.tile_pool(name="small", bufs=4) as small_pool,
        ):
            identity = persist.tile([128, 128], f32)
            make_identity(nc, identity)
            eps_t = persist.tile([128, 1], f32, tag="eps")
            nc.vector.memset(eps_t, LN_EPS)
            one_c = persist.tile([1, 1], f32, tag="onec")
            nc.vector.memset(one_c, 1.0)

            # ---- contiguous loads + PE transposes -> qT/kT [64, 2048] bf16
            # raw partition p holds rows 16p..16p+15; transpose slice j has
            # rows (2j, 2j+1); qT column = 16c + 8g + 2j + r
            qT = persist.tile([64, L], bf16, tag="qT")
            kT = persist.tile([64, L], bf16, tag="kT")
            for src, dstT in ((q_d, qT), (k_d, kT)):
                raw = raw_pool.tile([128, NTILES * D], f32, tag="raw")
                nc.sync.dma_start(
                    out=raw,
                    in_=src.ap().rearrange("(p r) d -> p (r d)", p=128),
                )
                d5 = dstT.rearrange("d (c g j r) -> d c g j r", g=2, j=4, r=2)
                for g in range(2):
                    pt = stage_pool.tile([128, 512], f32, tag="stage")
                    for j in range(4):
                        nc.tensor.transpose(
                            pt[:, j * 128:(j + 1) * 128],
                            raw[:, (4 * g + j) * 128:(4 * g + j + 1) * 128],
                            identity,
                        )
                    ptv = pt.rearrange("p (j c) -> p j c", c=128)
                    nc.vector.tensor_copy(
                        d5[:, :, g, :, 0].rearrange("d c j -> d j c"),
                        ptv[0:64],
                    )
                    nc.vector.tensor_copy(
                        d5[:, :, g, :, 1].rearrange("d c j -> d j c"),
                        ptv[64:128],
                    )

            # ---- v with ones column: [128, 16, 65] bf16 (tile-major load)
            vraw = raw_pool.tile([128, NTILES, D], f32, tag="raw")
            nc.sync.dma_start(
                out=vraw, in_=v_d.ap().rearrange("(t p) d -> p t d", p=128)
            )
            v_sb = persist.tile([128, NTILES, D + 1], bf16, tag="v")
            nc.scalar.copy(v_sb[:, :, 0:D], vraw)
            nc.vector.memset(v_sb[:, :, D:D + 1], 1.0)

            # ---- fc_wT augmented [65, 1024] bf16, row 64 = fc_b ----
            # contiguous: raw partition p = 8 rows; fcwT col = 8c + 2j + r
            fcwT = persist.tile([65, DM], bf16, tag="fcw")
            fraw = raw_pool.tile([128, (DM // 128) * D], f32, tag="raw")
            nc.sync.dma_start(
                out=fraw, in_=fcw_d.ap().rearrange("(p r) d -> p (r d)", p=128)
            )
            f5 = fcwT[0:64, :].rearrange("d (c j r) -> d c j r", j=4, r=2)
            pt = stage_pool.tile([128, 512], f32, tag="stage")
            for j in range(4):
                nc.tensor.transpose(
                    pt[:, j * 128:(j + 1) * 128],
                    fraw[:, j * 128:(j + 1) * 128],
                    identity,
                )
            ptv = pt.rearrange("p (j c) -> p j c", c=128)
            nc.vector.tensor_copy(f5[:, :, :, 0].rearrange("d c j -> d j c"),
                                  ptv[0:64])
            nc.vector.tensor_copy(f5[:, :, :, 1].rearrange("d c j -> d j c"),
                                  ptv[64:128])
            fcb_t = small_pool.tile([1, DM], f32, tag="fcb")
            nc.sync.dma_start(
                out=fcb_t,
                in_=bass.AP(tensor=fcb_d, offset=0, ap=[[0, 1], [1, DM]]),
            )
            nc.vector.tensor_copy(fcwT[64:65, :], fcb_t)

            if affine:
                gam_bc = persist.tile([128, DM], f32, tag="gam")
                bet_bc = persist.tile([128, DM], f32, tag="bet")
                nc.sync.dma_start(
                    out=gam_bc,
                    in_=bass.AP(tensor=gam_d, offset=0, ap=[[0, 128], [1, DM]]),
                )
                nc.sync.dma_start(
                    out=bet_bc,
                    in_=bass.AP(tensor=bet_d, offset=0, ap=[[0, 128], [1, DM]]),
                )

            kT2 = kT.rearrange("d (g two c) -> d g two c", two=2, c=128)

            state = {}

            def attention(s):
                qs = qT[:, s * 512:(s + 1) * 512]
                out_aug = pv_pool.tile([65, 512], f32, tag="pv")
                for g in range(NTILES // 2):
                    st = stage_pool.tile([128, 1024], f32, tag="stage")
                    nc.tensor.matmul(st[:, 0:512], kT2[:, g, 0, :], qs,
                                     start=True, stop=True)
                    nc.tensor.matmul(st[:, 512:1024], kT2[:, g, 1, :], qs,
                                     start=True, stop=True)
                    et = et_pool.tile([128, 1024], bf16, tag="et")
                    nc.scalar.activation(
                        out=et, in_=st,
                        func=mybir.ActivationFunctionType.Exp, scale=SCALE,
                    )
                    nc.tensor.matmul(out_aug, v_sb[:, 2 * g, :], et[:, 0:512],
                                     start=(g == 0), stop=False)
                    nc.tensor.matmul(out_aug, v_sb[:, 2 * g + 1, :],
                                     et[:, 512:1024],
                                     start=False, stop=(g == NTILES // 2 - 1))
                return out_aug

            def dance(s, out_aug):
                # denom row -> per-partition [128, 4] via 4 tiny PE matmuls
                drow = small_pool.tile([1, 512], f32, tag="drow")
                nc.scalar.copy(drow, out_aug[64:65, :])
                dT = stage_pool.tile([128, 4], f32, tag="stage")
                for t in range(4):
                    nc.tensor.matmul(dT[:, t:t + 1],
                                     drow[:, t * 128:(t + 1) * 128], one_c,
                                     start=True, stop=True)
                rT = small_pool.tile([128, 4], f32, tag="rT")
                nc.vector.reciprocal(rT, dT)
                nc.sync.dma_start(
                    out=bass.AP(tensor=recip_s, offset=s * 512,
                                ap=[[1, 128], [128, 4]]),
                    in_=rT,
                )
                rbc = norm_pool.tile([65, 512], f32, tag="rbc")
                nc.sync.dma_start(
                    out=rbc,
                    in_=bass.AP(tensor=recip_s, offset=s * 512,
                                ap=[[0, 65], [1, 512]]),
                )
                # normalize + evacuate PSUM + cast bf16; row 64 -> exactly 1.0
                outT = norm_pool.tile([65, 512], bf16, tag="outT")
                nc.vector.tensor_mul(outT, out_aug, rbc)
                state[s] = {"outT": outT}

            def epilogue(s):
                outT = state[s]["outT"]
                mv_all = small_pool.tile([128, 4, 2], f32, tag="mv")
                x_ts = []
                for pi in range(4):
                    t = s * 4 + pi
                    fc_ps = fc_pool.tile([128, DM], f32, tag="fc")
                    lhs = outT[:, pi * 128:(pi + 1) * 128]
                    nc.tensor.matmul(fc_ps[:, 0:512], lhs, fcwT[:, 0:512],
                                     start=True, stop=True)
                    nc.tensor.matmul(fc_ps[:, 512:1024], lhs,
                                     fcwT[:, 512:1024],
                                     start=True, stop=True)
                    res_t = res_pool.tile([128, DM], f32, tag="res")
                    nc.sync.dma_start(
                        out=res_t, in_=res_d[t * 128:(t + 1) * 128, :]
                    )
                    x_t = x_pool.tile([128, DM], f32, tag="x")
                    nc.vector.tensor_add(x_t, fc_ps, res_t)
                    x_ts.append(x_t)
                    stats = small_pool.tile([128, 2, 6], f32, tag="stats")
                    nc.vector.bn_stats(out=stats[:, 0, :], in_=x_t[:, 0:512])
                    nc.vector.bn_stats(out=stats[:, 1, :],
                                       in_=x_t[:, 512:1024])
                    nc.vector.bn_aggr(out=mv_all[:, pi, :], in_=stats)

                # batched rsqrt: rstd = exp(-0.5*ln(var+eps))
                rstd4 = small_pool.tile([128, 4], f32, tag="rstd")
                nc.scalar.activation(
                    out=rstd4, in_=mv_all[:, :, 1],
                    func=mybir.ActivationFunctionType.Ln, bias=eps_t,
                )
                nc.scalar.activation(
                    out=rstd4, in_=rstd4,
                    func=mybir.ActivationFunctionType.Exp, scale=-0.5,
                )

                nm4 = small_pool.tile([128, 4], f32, tag="nm")
                nc.vector.tensor_tensor(
                    out=nm4, in0=mv_all[:, :, 0], in1=rstd4,
                    op=mybir.AluOpType.mult,
                )
                nc.vector.tensor_scalar_mul(out=nm4, in0=nm4, scalar1=-1.0)
                for pi in range(4):
                    t = s * 4 + pi
                    out_t = out_pool.tile([128, DM], f32, tag="out")
                    if pi % 2 == 0:
                        nc.vector.tensor_scalar(
                            out=out_t, in0=x_ts[pi],
                            scalar1=mv_all[:, pi, 0:1],
                            scalar2=rstd4[:, pi:pi + 1],
                            op0=mybir.AluOpType.subtract,
                            op1=mybir.AluOpType.mult,
                        )
                    else:
                        nc.scalar.activation(
                            out=out_t, in_=x_ts[pi],
                            func=mybir.ActivationFunctionType.Identity,
                            bias=nm4[:, pi:pi + 1],
                            scale=rstd4[:, pi:pi + 1],
                        )
                    if affine:
                        nc.vector.tensor_mul(out_t, out_t, gam_bc)
                        nc.vector.tensor_add(out_t, out_t, bet_bc)
                    nc.sync.dma_start(
                        out=out_d[t * 128:(t + 1) * 128, :], in_=out_t
                    )
                del state[s]

            # software pipeline: epilogue runs one slice behind attention so
            # the strict-FIFO PE queue never waits on the denominator dance
            for s in range(NSLICES + 1):
                if s < NSLICES:
                    oa = attention(s)
                if s >= 1:
                    epilogue(s - 1)
                if s < NSLICES:
                    dance(s, oa)

    nc.finalize()
    return nc


LAST_RESULTS = None


def kernel(q, k, v, residual, fc_w, fc_b, ln_gamma, ln_beta):
    from concourse.bass_utils import run_bass_kernel_spmd

    global LAST_RESULTS
    affine = not (
        np.allclose(ln_gamma, 1.0) and np.allclose(ln_beta, 0.0)
    )
    key = ("v4", affine)
    if key not in _CACHE:
        _CACHE[key] = _build(affine)
    nc = _CACHE[key]

    q = np.ascontiguousarray(q, dtype=np.float32)
    k = np.ascontiguousarray(k, dtype=np.float32)
    v = np.ascontiguousarray(v, dtype=np.float32)
    residual = np.ascontiguousarray(residual, dtype=np.float32)
    fc_w = np.ascontiguousarray(fc_w, dtype=np.float32)
    fc_b = np.ascontiguousarray(fc_b, dtype=np.float32)
    ln_gamma = np.ascontiguousarray(ln_gamma, dtype=np.float32)
    ln_beta = np.ascontiguousarray(ln_beta, dtype=np.float32)

    in_maps = [
        {
            "q": q[b], "k": k[b], "v": v[b], "residual": residual[b],
            "fc_w": fc_w, "fc_b": fc_b,
            "ln_gamma": ln_gamma, "ln_beta": ln_beta,
        }
        for b in range(B)
    ]
    res = run_bass_kernel_spmd(nc, in_maps, core_ids=list(range(B)))
    LAST_RESULTS = res
    return np.stack([res.results[b]["out"] for b in range(B)], axis=0)
